# Initial kernel scaffold
#
"""MiniAttentionQHead Trainium2 kernel (8-core data parallel).

Algorithm (algebraically identical to the reference, computed without
materializing the scattered buffer or the full QKV):

  kv tokens per row b = [hidden, buf[0..7]] where buf[ptr] == hidden, so
  there are 8 distinct tokens (hidden + 7 untouched context slots) and
  hidden's softmax term counts twice.

  scores[b,h,j] = (hidden[b] @ Wq_h.T / sqrt(D)) . (tok_j[b] @ Wk_h.T)
  out[b]        = sum_j attn[b,h,j] * (tok_j[b] @ Wv_h.T)  -> @ w_out.T + b_out

  The output only needs A=2 channels, so Wv and w_out fold into
  U[(2h+a), :] = sum_d w_out[a, h*D+d] * Wv[h*D+d, :]  (host precompute),
  and the v-side per token reduces to  vproj[b, j, (2h+a)] = tok_j[b] . U[(2h+a)].

Per core (512 rows): q-proj, 8 k-projections and vproj run on the PE in
float32r (full fp32 data, reduced-precision matmul at full PE rate), dots
q.k on the DVE, softmax on DVE+ACT.  Heads are split into two passes of 8
so Wk/Wq tiles fit SBUF.  All activations/weights are transposed on the
host so no on-device transposes are needed.
"""

import math

import numpy as np

B, H, NH, W, A = 4096, 2048, 16, 8, 2
D = H // NH  # 128
NCORES = 8
R = B // NCORES  # 512 rows per core
NT = R // 128  # 4 row tiles
KC = H // 128  # 16 contraction chunks
PASSES = 2
HPP = NH // PASSES  # 8 heads per pass
CW = HPP * D  # 1024 channels per pass
NTOK = W  # 8 distinct kv tokens (hidden + 7 ctx)

_cache = {}


def _patch_tile_framework():
    """This environment's walrus accepts only ONE semaphore wait per
    instruction; Tile attaches several.  Patch the end-of-kernel drain and
    add a post-pass that hoists excess waits onto preceding same-engine
    NOPs (engine queues execute sequentially, so semantics are identical).
    """
    import concourse.tile as tile
    from concourse import mybir
    from concourse.vector_clock import ScopedClock

    if getattr(tile.TileContext, "_ant_drain_patched", False):
        return

    def patched(self, tick_clock, wait_clock):
        drain_inst = self.nc.sync.drain()
        wait_clock.add_sem_waits(
            drain_inst.ins, ScopedClock({None: tick_clock.global_clock})
        )
        si = drain_inst.ins.sync_info
        waits = list(si.on_wait or [])
        if len(waits) > 1:
            si.on_wait = waits[:1]
            for w in waits[1:]:
                extra = self.nc.sync.drain()
                extra.ins.sync_info = mybir.SyncInfo(on_wait=[w], on_update=[])
        self.nc.all_engine_barrier()
        assert self.sems is not None
        popped = self.nc._tile_sem_poison_stack.pop()
        assert popped is self._sem_poison
        self.nc.clear_and_free_semaphores(list(self.sems.allocated().values()))
        self.nc.all_engine_barrier()

    tile.TileContext._drain_and_barrier = patched
    tile.TileContext._ant_drain_patched = True


def _split_waits(nc, max_waits=1):
    from concourse import mybir

    cnt = 0
    for fn in nc.m.functions:
        for bb in fn.blocks:
            changed = False
            out = []
            for inst in bb.instructions:
                si = inst.sync_info
                if si is not None:
                    waits = list(si.on_wait or [])
                    if len(waits) > max_waits:
                        extra = waits[:-max_waits]
                        for k in range(0, len(extra), max_waits):
                            nop = mybir.InstNoOp(
                                name=f"I-antws-{cnt}", ins=[], outs=[]
                            )
                            cnt += 1
                            nop.engine = inst.engine
                            nop.sync_info = mybir.SyncInfo(
                                on_wait=extra[k : k + max_waits], on_update=[]
                            )
                            out.append(nop)
                        inst.sync_info = mybir.SyncInfo(
                            on_wait=waits[-max_waits:],
                            on_update=list(si.on_update or []),
                        )
                        changed = True
                out.append(inst)
            if changed:
                bb.instructions = out


def _build_nc(reps=1):
    key = ("nc", reps)
    if key in _cache:
        return _cache[key]

    import concourse.bass as bass
    import concourse.tile as tile
    from concourse import mybir

    _patch_tile_framework()

    f32 = mybir.dt.float32
    f32r = mybir.dt.float32r
    X = mybir.AxisListType.X
    XY = mybir.AxisListType.XY
    ADD = mybir.AluOpType.add
    MAX = mybir.AluOpType.max

    nc = bass.Bass(target_bir_lowering=False)

    hid_d = nc.dram_tensor("hidT", [KC, 128, R], f32, kind="ExternalInput")
    ctx_d = nc.dram_tensor("ctxT", [W - 1, KC, 128, R], f32, kind="ExternalInput")
    wq_d = nc.dram_tensor("wqT", [KC, 128, H], f32, kind="ExternalInput")
    wk_d = nc.dram_tensor("wkT", [KC, 128, H], f32, kind="ExternalInput")
    u_d = nc.dram_tensor("uT", [KC, 128, 2 * NH], f32, kind="ExternalInput")
    out_d = nc.dram_tensor("qout", [R, A], f32, kind="ExternalOutput")

    qscale = 1.0 / math.sqrt(D)

    with tile.TileContext(nc) as tc:
        with tc.tile_pool(name="outer", bufs=1) as outer:
            hid_sb = outer.tile([128, KC, R], f32r, tag="hidT")
            # [c, p, r] -> [p, c, r]
            nc.sync.dma_start(
                out=hid_sb, in_=hid_d[:, :, :].rearrange("c p r -> p c r").bitcast(f32r)
            )
            out_sbs = [
                outer.tile([128, A], f32, tag=f"out{t}", name=f"out{t}")
                for t in range(NT)
            ]

            for _rep in range(reps):
              for pp in range(PASSES):
                with (
                    tc.tile_pool(name=f"res{pp}", bufs=1) as res,
                    tc.tile_pool(name=f"wqs{pp}", bufs=4) as wqs,
                ):
                    # pass-resident tiles
                    wk_sb = res.tile([128, KC, CW], f32r, tag="wk")
                    for c4 in range(4):  # 4 DMAs for overlap
                        nc.sync.dma_start(
                            out=wk_sb[:, 4 * c4 : 4 * c4 + 4, :],
                            in_=wk_d[4 * c4 : 4 * c4 + 4, :, pp * CW : (pp + 1) * CW]
                            .rearrange("c p n -> p c n")
                            .bitcast(f32r),
                        )
                    u_sb = res.tile([128, KC, NH], f32r, tag="u")
                    nc.sync.dma_start(
                        out=u_sb,
                        in_=u_d[:, :, pp * NH : (pp + 1) * NH]
                        .rearrange("c p m -> p c m")
                        .bitcast(f32r),
                    )
                    q_sbs = [
                        res.tile([128, CW], f32, tag=f"q{t}", name=f"q{t}")
                        for t in range(NT)
                    ]
                    sc_sbs = [
                        res.tile([128, HPP, NTOK], f32, tag=f"sc{t}", name=f"sc{t}")
                        for t in range(NT)
                    ]
                    vp_sbs = [
                        res.tile(
                            [128, NTOK, 2 * HPP], f32, tag=f"vp{t}", name=f"vp{t}"
                        )
                        for t in range(NT)
                    ]

                    # ---- Q phase: q = hidden @ Wq.T (this pass's head half)
                    qps_ctx = tc.tile_pool(name=f"qps{pp}", bufs=NT, space="PSUM")
                    qps = qps_ctx.__enter__()
                    q_ps = [
                        qps.tile([128, CW], f32, tag="qps", name=f"qps{t}")
                        for t in range(NT)
                    ]
                    for c in range(KC):
                        wq_sb = wqs.tile([128, CW], f32r, tag="wq")
                        nc.sync.dma_start(
                            out=wq_sb,
                            in_=wq_d[c, :, pp * CW : (pp + 1) * CW].bitcast(f32r),
                        )
                        for t in range(NT):
                            lhs = hid_sb[:, c, t * 128 : (t + 1) * 128]
                            for b in range(CW // 512):
                                nc.tensor.matmul(
                                    q_ps[t][:, b * 512 : (b + 1) * 512],
                                    lhs,
                                    wq_sb[:, b * 512 : (b + 1) * 512],
                                    start=(c == 0),
                                    stop=(c == KC - 1),
                                )
                    for t in range(NT):
                        # PSUM -> SBUF, folding in the 1/sqrt(D) score scale
                        nc.scalar.activation(
                            out=q_sbs[t],
                            in_=q_ps[t],
                            func=mybir.ActivationFunctionType.Copy,
                            scale=qscale,
                        )
                    qps_ctx.__exit__(None, None, None)

                    # ---- KV phase: per (token, tile): k-proj + vproj + dots
                    with (
                        tc.tile_pool(name=f"ctx{pp}", bufs=4) as ctxp,
                        tc.tile_pool(name=f"prod{pp}", bufs=3) as prodp,
                        tc.tile_pool(name=f"kvps{pp}", bufs=2, space="PSUM") as kvps,
                    ):
                        for j in range(NTOK):
                            for t in range(NT):
                                if j == 0:
                                    tok = None
                                else:
                                    tok = ctxp.tile([128, KC, 128], f32r, tag="ctx")
                                    nc.sync.dma_start(
                                        out=tok,
                                        in_=ctx_d[
                                            j - 1, :, :, t * 128 : (t + 1) * 128
                                        ]
                                        .rearrange("c p r -> p c r")
                                        .bitcast(f32r),
                                    )
                                kb0 = kvps.tile([128, 512], f32, tag="kb0")
                                kb1 = kvps.tile([128, 512], f32, tag="kb1")
                                vpp = kvps.tile([128, 2 * HPP], f32, tag="vpp")
                                for c in range(KC):
                                    lhs = (
                                        hid_sb[:, c, t * 128 : (t + 1) * 128]
                                        if j == 0
                                        else tok[:, c, :]
                                    )
                                    st = c == 0
                                    sp = c == KC - 1
                                    nc.tensor.matmul(
                                        kb0, lhs, wk_sb[:, c, 0:512], start=st, stop=sp
                                    )
                                    nc.tensor.matmul(
                                        kb1, lhs, wk_sb[:, c, 512:1024], start=st, stop=sp
                                    )
                                    nc.tensor.matmul(
                                        vpp, lhs, u_sb[:, c, :], start=st, stop=sp
                                    )
                                pr0 = prodp.tile([128, 512], f32, tag="pr0")
                                pr1 = prodp.tile([128, 512], f32, tag="pr1")
                                nc.vector.tensor_mul(pr0, kb0, q_sbs[t][:, 0:512])
                                nc.vector.tensor_mul(pr1, kb1, q_sbs[t][:, 512:1024])
                                nc.vector.tensor_reduce(
                                    out=sc_sbs[t][:, 0 : HPP // 2, j],
                                    in_=pr0.rearrange("p (h d) -> p h d", d=D),
                                    axis=X,
                                    op=ADD,
                                )
                                nc.vector.tensor_reduce(
                                    out=sc_sbs[t][:, HPP // 2 : HPP, j],
                                    in_=pr1.rearrange("p (h d) -> p h d", d=D),
                                    axis=X,
                                    op=ADD,
                                )
                                nc.scalar.activation(
                                    out=vp_sbs[t][:, j, :],
                                    in_=vpp,
                                    func=mybir.ActivationFunctionType.Copy,
                                )

                    # ---- softmax + combine per tile
                    with tc.tile_pool(name=f"sm{pp}", bufs=2) as smp:
                        for t in range(NT):
                            mx = smp.tile([128, HPP], f32, tag=f"m{t}")
                            nc.vector.tensor_reduce(
                                out=mx, in_=sc_sbs[t], axis=X, op=MAX
                            )
                            et = smp.tile([128, HPP, NTOK], f32, tag=f"e{t}")
                            for j in range(NTOK):
                                nc.vector.tensor_sub(
                                    et[:, :, j], sc_sbs[t][:, :, j], mx
                                )
                            nc.scalar.activation(
                                out=et, in_=et, func=mybir.ActivationFunctionType.Exp
                            )
                            s8 = smp.tile([128, HPP], f32, tag=f"s8{t}")
                            nc.vector.tensor_reduce(out=s8, in_=et, axis=X, op=ADD)
                            # hidden token appears twice in the kv list
                            nc.vector.tensor_add(s8, s8, et[:, :, 0])
                            rcp = smp.tile([128, HPP], f32, tag=f"r{t}")
                            nc.vector.reciprocal(rcp, s8)
                            at = smp.tile([128, HPP, NTOK], f32, tag=f"a{t}")
                            for j in range(NTOK):
                                nc.vector.tensor_mul(at[:, :, j], et[:, :, j], rcp)
                            vv = vp_sbs[t].rearrange("p j (h a) -> p h j a", a=A)
                            for a in range(A):
                                tmp = smp.tile([128, HPP, NTOK], f32, tag=f"tm{t}")
                                nc.vector.tensor_mul(tmp, at, vv[:, :, :, a])
                                r1 = smp.tile([128, 1], f32, tag=f"r1{t}")
                                r2 = smp.tile([128, 1], f32, tag=f"r2{t}")
                                nc.vector.tensor_reduce(
                                    out=r1, in_=tmp, axis=XY, op=ADD
                                )
                                nc.vector.tensor_reduce(
                                    out=r2, in_=tmp[:, :, 0], axis=X, op=ADD
                                )
                                nc.vector.tensor_add(r1, r1, r2)
                                if pp == 0:
                                    nc.vector.tensor_copy(
                                        out=out_sbs[t][:, a : a + 1], in_=r1
                                    )
                                else:
                                    nc.vector.tensor_add(
                                        out_sbs[t][:, a : a + 1],
                                        out_sbs[t][:, a : a + 1],
                                        r1,
                                    )

            for t in range(NT):
                nc.sync.dma_start(
                    out=out_d[t * 128 : (t + 1) * 128, :], in_=out_sbs[t]
                )

    _split_waits(nc)
    _cache[key] = nc
    return nc


def _prep_inputs(hidden_state, context_buffer, w_qkv, w_out, b_out, context_ptr):
    """Host-side sharding + layout (transposes, weight folding)."""
    hidden_state = np.ascontiguousarray(hidden_state, dtype=np.float32)
    context_buffer = np.ascontiguousarray(context_buffer, dtype=np.float32)
    w_qkv = np.ascontiguousarray(w_qkv, dtype=np.float32)
    w_out = np.ascontiguousarray(w_out, dtype=np.float32)

    ptr = int(context_ptr) % W
    kept = [w for w in range(W) if w != ptr]

    wqT = np.ascontiguousarray(w_qkv[0:H, :].T).reshape(KC, 128, H)
    wkT = np.ascontiguousarray(w_qkv[H : 2 * H, :].T).reshape(KC, 128, H)
    # U[(2h+a), ci] = sum_d w_out[a, h*D+d] * Wv[h*D+d, ci]
    wo = w_out.reshape(A, NH, D)
    wv = w_qkv[2 * H : 3 * H, :].reshape(NH, D, H)
    U = np.einsum("ahd,hdc->hac", wo, wv, optimize=True).reshape(2 * NH, H)
    uT = np.ascontiguousarray(U.T).reshape(KC, 128, 2 * NH)

    in_maps = []
    for c in range(NCORES):
        rows = slice(c * R, (c + 1) * R)
        hidT = np.ascontiguousarray(hidden_state[rows].T).reshape(KC, 128, R)
        ctx = context_buffer[rows][:, kept, :]  # [R, 7, H]
        ctxT = np.ascontiguousarray(ctx.transpose(1, 2, 0)).reshape(
            W - 1, KC, 128, R
        )
        in_maps.append(
            dict(hidT=hidT, ctxT=ctxT, wqT=wqT, wkT=wkT, uT=uT)
        )
    return in_maps


def kernel(hidden_state, context_buffer, w_qkv, w_out, b_out, context_ptr):
    from concourse.bass_utils import run_bass_kernel_spmd

    nc = _build_nc()
    in_maps = _prep_inputs(
        hidden_state, context_buffer, w_qkv, w_out, b_out, context_ptr
    )
    res = run_bass_kernel_spmd(nc, in_maps, core_ids=list(range(NCORES)))
    out = np.concatenate([r["qout"] for r in res.results], axis=0)
    return (out + np.asarray(b_out, dtype=np.float32)[None, :]).astype(np.float32)



# revision 1
# speedup vs baseline: 1.0657x; 1.0657x over previous
"""MiniAttentionQHead Trainium2 kernel (8-core data parallel).

Algorithm (algebraically identical to the reference, computed without
materializing the scattered buffer or the full QKV):

  kv tokens per row b = [hidden, buf[0..7]] where buf[ptr] == hidden, so
  there are 8 distinct tokens (hidden + 7 untouched context slots) and
  hidden's softmax term counts twice.

  scores[b,h,j] = (hidden[b] @ Wq_h.T / sqrt(D)) . (tok_j[b] @ Wk_h.T)
  out[b]        = sum_j attn[b,h,j] * (tok_j[b] @ Wv_h.T)  -> @ w_out.T + b_out

  The output only needs A=2 channels, so Wv and w_out fold into
  U[(2h+a), :] = sum_d w_out[a, h*D+d] * Wv[h*D+d, :]  (host precompute),
  and the v-side per token reduces to  vproj[b, j, (2h+a)] = tok_j[b] . U[(2h+a)].

Per core (512 rows): q-proj, 8 k-projections and vproj run on the PE in
float32r (full fp32 data, reduced-precision matmul at full PE rate), dots
q.k on the DVE, softmax on DVE+ACT.  Heads are split into two passes of 8
so Wk/Wq tiles fit SBUF.  All activations/weights are transposed on the
host so no on-device transposes are needed.
"""

import math

import numpy as np

B, H, NH, W, A = 4096, 2048, 16, 8, 2
D = H // NH  # 128
NCORES = 8
R = B // NCORES  # 512 rows per core
NT = R // 128  # 4 row tiles
KC = H // 128  # 16 contraction chunks
PASSES = 2
HPP = NH // PASSES  # 8 heads per pass
CW = HPP * D  # 1024 channels per pass
NTOK = W  # 8 distinct kv tokens (hidden + 7 ctx)

_cache = {}


def _patch_tile_framework():
    """This environment's walrus accepts only ONE semaphore wait per
    instruction; Tile attaches several.  Patch the end-of-kernel drain and
    add a post-pass that hoists excess waits onto preceding same-engine
    NOPs (engine queues execute sequentially, so semantics are identical).
    """
    import concourse.tile as tile
    from concourse import mybir
    from concourse.vector_clock import ScopedClock

    if getattr(tile.TileContext, "_ant_drain_patched", False):
        return

    def patched(self, tick_clock, wait_clock):
        drain_inst = self.nc.sync.drain()
        wait_clock.add_sem_waits(
            drain_inst.ins, ScopedClock({None: tick_clock.global_clock})
        )
        si = drain_inst.ins.sync_info
        waits = list(si.on_wait or [])
        if len(waits) > 1:
            si.on_wait = waits[:1]
            for w in waits[1:]:
                extra = self.nc.sync.drain()
                extra.ins.sync_info = mybir.SyncInfo(on_wait=[w], on_update=[])
        self.nc.all_engine_barrier()
        assert self.sems is not None
        popped = self.nc._tile_sem_poison_stack.pop()
        assert popped is self._sem_poison
        self.nc.clear_and_free_semaphores(list(self.sems.allocated().values()))
        self.nc.all_engine_barrier()

    tile.TileContext._drain_and_barrier = patched
    tile.TileContext._ant_drain_patched = True


def _split_waits(nc, max_waits=1):
    from concourse import mybir

    cnt = 0
    for fn in nc.m.functions:
        for bb in fn.blocks:
            changed = False
            out = []
            for inst in bb.instructions:
                si = inst.sync_info
                if si is not None:
                    waits = list(si.on_wait or [])
                    if len(waits) > max_waits:
                        extra = waits[:-max_waits]
                        for k in range(0, len(extra), max_waits):
                            nop = mybir.InstNoOp(
                                name=f"I-antws-{cnt}", ins=[], outs=[]
                            )
                            cnt += 1
                            nop.engine = inst.engine
                            nop.sync_info = mybir.SyncInfo(
                                on_wait=extra[k : k + max_waits], on_update=[]
                            )
                            out.append(nop)
                        inst.sync_info = mybir.SyncInfo(
                            on_wait=waits[-max_waits:],
                            on_update=list(si.on_update or []),
                        )
                        changed = True
                out.append(inst)
            if changed:
                bb.instructions = out


def _build_nc(reps=1):
    key = ("nc", reps)
    if key in _cache:
        return _cache[key]

    import concourse.bass as bass
    import concourse.tile as tile
    from concourse import mybir

    _patch_tile_framework()

    f32 = mybir.dt.float32
    f32r = mybir.dt.float32r
    X = mybir.AxisListType.X
    XY = mybir.AxisListType.XY
    ADD = mybir.AluOpType.add
    MAX = mybir.AluOpType.max

    nc = bass.Bass(target_bir_lowering=False)

    hid_d = nc.dram_tensor("hidT", [KC, 128, R], f32, kind="ExternalInput")
    ctx_d = nc.dram_tensor("ctxT", [W - 1, KC, 128, R], f32, kind="ExternalInput")
    wq_d = nc.dram_tensor("wqT", [KC, 128, H], f32, kind="ExternalInput")
    wk_d = nc.dram_tensor("wkT", [KC, 128, H], f32, kind="ExternalInput")
    u_d = nc.dram_tensor("uT", [KC, 128, 2 * NH], f32, kind="ExternalInput")
    out_d = nc.dram_tensor("qout", [R, A], f32, kind="ExternalOutput")

    qscale = 1.0 / math.sqrt(D)

    with tile.TileContext(nc) as tc:
        with tc.tile_pool(name="outer", bufs=1) as outer:
            hid_sb = outer.tile([128, KC, R], f32r, tag="hidT")
            # [c, p, r] -> [p, c, r]
            nc.sync.dma_start(
                out=hid_sb, in_=hid_d[:, :, :].rearrange("c p r -> p c r").bitcast(f32r)
            )
            out_sbs = [
                outer.tile([128, A], f32, tag=f"out{t}", name=f"out{t}")
                for t in range(NT)
            ]

            for _rep in range(reps):
              for pp in range(PASSES):
                with (
                    tc.tile_pool(name=f"res{pp}", bufs=1) as res,
                    tc.tile_pool(name=f"wqs{pp}", bufs=4) as wqs,
                ):
                    # pass-resident tiles
                    wk_sb = res.tile([128, KC, CW], f32r, tag="wk")
                    for c4 in range(4):  # 4 DMAs for overlap
                        nc.sync.dma_start(
                            out=wk_sb[:, 4 * c4 : 4 * c4 + 4, :],
                            in_=wk_d[4 * c4 : 4 * c4 + 4, :, pp * CW : (pp + 1) * CW]
                            .rearrange("c p n -> p c n")
                            .bitcast(f32r),
                        )
                    u_sb = res.tile([128, KC, NH], f32r, tag="u")
                    nc.sync.dma_start(
                        out=u_sb,
                        in_=u_d[:, :, pp * NH : (pp + 1) * NH]
                        .rearrange("c p m -> p c m")
                        .bitcast(f32r),
                    )
                    q_sbs = [
                        res.tile([128, CW], f32, tag=f"q{t}", name=f"q{t}")
                        for t in range(NT)
                    ]
                    sc_sbs = [
                        res.tile([128, HPP, NTOK], f32, tag=f"sc{t}", name=f"sc{t}")
                        for t in range(NT)
                    ]
                    vp_sbs = [
                        res.tile(
                            [128, NTOK, 2 * HPP], f32, tag=f"vp{t}", name=f"vp{t}"
                        )
                        for t in range(NT)
                    ]

                    # ---- Q phase: q = hidden @ Wq.T (this pass's head half)
                    qps_ctx = tc.tile_pool(name=f"qps{pp}", bufs=NT, space="PSUM")
                    qps = qps_ctx.__enter__()
                    q_ps = [
                        qps.tile([128, CW], f32, tag="qps", name=f"qps{t}")
                        for t in range(NT)
                    ]
                    for c in range(KC):
                        wq_sb = wqs.tile([128, CW], f32r, tag="wq")
                        nc.sync.dma_start(
                            out=wq_sb,
                            in_=wq_d[c, :, pp * CW : (pp + 1) * CW].bitcast(f32r),
                        )
                        for t in range(NT):
                            lhs = hid_sb[:, c, t * 128 : (t + 1) * 128]
                            for b in range(CW // 512):
                                nc.tensor.matmul(
                                    q_ps[t][:, b * 512 : (b + 1) * 512],
                                    lhs,
                                    wq_sb[:, b * 512 : (b + 1) * 512],
                                    start=(c == 0),
                                    stop=(c == KC - 1),
                                )
                    for t in range(NT):
                        # PSUM -> SBUF, folding in the 1/sqrt(D) score scale
                        nc.scalar.activation(
                            out=q_sbs[t],
                            in_=q_ps[t],
                            func=mybir.ActivationFunctionType.Copy,
                            scale=qscale,
                        )
                    qps_ctx.__exit__(None, None, None)

                    # ---- KV phase: per (token, tile): k-proj + vproj + dots
                    with (
                        tc.tile_pool(name=f"ctx{pp}", bufs=4) as ctxp,
                        tc.tile_pool(name=f"prod{pp}", bufs=3) as prodp,
                        tc.tile_pool(name=f"kvps{pp}", bufs=2, space="PSUM") as kvps,
                    ):
                        for j in range(NTOK):
                            for t in range(NT):
                                if j == 0:
                                    tok = None
                                else:
                                    tok = ctxp.tile([128, KC, 128], f32r, tag="ctx")
                                    nc.sync.dma_start(
                                        out=tok,
                                        in_=ctx_d[
                                            j - 1, :, :, t * 128 : (t + 1) * 128
                                        ]
                                        .rearrange("c p r -> p c r")
                                        .bitcast(f32r),
                                    )
                                kb0 = kvps.tile([128, 512], f32, tag="kb0")
                                kb1 = kvps.tile([128, 512], f32, tag="kb1")
                                vpp = kvps.tile([128, 2 * HPP], f32, tag="vpp")
                                for c in range(KC):
                                    lhs = (
                                        hid_sb[:, c, t * 128 : (t + 1) * 128]
                                        if j == 0
                                        else tok[:, c, :]
                                    )
                                    st = c == 0
                                    sp = c == KC - 1
                                    nc.tensor.matmul(
                                        kb0, lhs, wk_sb[:, c, 0:512], start=st, stop=sp
                                    )
                                    nc.tensor.matmul(
                                        kb1, lhs, wk_sb[:, c, 512:1024], start=st, stop=sp
                                    )
                                    nc.tensor.matmul(
                                        vpp, lhs, u_sb[:, c, :], start=st, stop=sp
                                    )
                                pr0 = prodp.tile([128, 512], f32, tag="pr0")
                                pr1 = prodp.tile([128, 512], f32, tag="pr1")
                                nc.vector.tensor_mul(pr0, kb0, q_sbs[t][:, 0:512])
                                nc.vector.tensor_mul(pr1, kb1, q_sbs[t][:, 512:1024])
                                nc.vector.tensor_reduce(
                                    out=sc_sbs[t][:, 0 : HPP // 2, j],
                                    in_=pr0.rearrange("p (h d) -> p h d", d=D),
                                    axis=X,
                                    op=ADD,
                                )
                                nc.vector.tensor_reduce(
                                    out=sc_sbs[t][:, HPP // 2 : HPP, j],
                                    in_=pr1.rearrange("p (h d) -> p h d", d=D),
                                    axis=X,
                                    op=ADD,
                                )
                                nc.scalar.activation(
                                    out=vp_sbs[t][:, j, :],
                                    in_=vpp,
                                    func=mybir.ActivationFunctionType.Copy,
                                )

                    # ---- softmax + combine per tile
                    with tc.tile_pool(name=f"sm{pp}", bufs=2) as smp:
                        for t in range(NT):
                            mx = smp.tile([128, HPP], f32, tag=f"m{t}")
                            nc.vector.tensor_reduce(
                                out=mx, in_=sc_sbs[t], axis=X, op=MAX
                            )
                            et = smp.tile([128, HPP, NTOK], f32, tag=f"e{t}")
                            for j in range(NTOK):
                                nc.vector.tensor_sub(
                                    et[:, :, j], sc_sbs[t][:, :, j], mx
                                )
                            nc.scalar.activation(
                                out=et, in_=et, func=mybir.ActivationFunctionType.Exp
                            )
                            s8 = smp.tile([128, HPP], f32, tag=f"s8{t}")
                            nc.vector.tensor_reduce(out=s8, in_=et, axis=X, op=ADD)
                            # hidden token appears twice in the kv list
                            nc.vector.tensor_add(s8, s8, et[:, :, 0])
                            rcp = smp.tile([128, HPP], f32, tag=f"r{t}")
                            nc.vector.reciprocal(rcp, s8)
                            at = smp.tile([128, HPP, NTOK], f32, tag=f"a{t}")
                            for j in range(NTOK):
                                nc.vector.tensor_mul(at[:, :, j], et[:, :, j], rcp)
                            vv = vp_sbs[t].rearrange("p j (h a) -> p h j a", a=A)
                            for a in range(A):
                                tmp = smp.tile([128, HPP, NTOK], f32, tag=f"tm{t}")
                                nc.vector.tensor_mul(tmp, at, vv[:, :, :, a])
                                r1 = smp.tile([128, 1], f32, tag=f"r1{t}")
                                r2 = smp.tile([128, 1], f32, tag=f"r2{t}")
                                nc.vector.tensor_reduce(
                                    out=r1, in_=tmp, axis=XY, op=ADD
                                )
                                nc.vector.tensor_reduce(
                                    out=r2, in_=tmp[:, :, 0], axis=X, op=ADD
                                )
                                nc.vector.tensor_add(r1, r1, r2)
                                if pp == 0:
                                    nc.vector.tensor_copy(
                                        out=out_sbs[t][:, a : a + 1], in_=r1
                                    )
                                else:
                                    nc.vector.tensor_add(
                                        out_sbs[t][:, a : a + 1],
                                        out_sbs[t][:, a : a + 1],
                                        r1,
                                    )

            for t in range(NT):
                nc.sync.dma_start(
                    out=out_d[t * 128 : (t + 1) * 128, :], in_=out_sbs[t]
                )

    _split_waits(nc)
    _cache[key] = nc
    return nc


def _prep_inputs(hidden_state, context_buffer, w_qkv, w_out, b_out, context_ptr):
    """Host-side sharding + layout (transposes, weight folding)."""
    hidden_state = np.ascontiguousarray(hidden_state, dtype=np.float32)
    context_buffer = np.ascontiguousarray(context_buffer, dtype=np.float32)
    w_qkv = np.ascontiguousarray(w_qkv, dtype=np.float32)
    w_out = np.ascontiguousarray(w_out, dtype=np.float32)

    ptr = int(context_ptr) % W
    kept = [w for w in range(W) if w != ptr]

    wqT = np.ascontiguousarray(w_qkv[0:H, :].T).reshape(KC, 128, H)
    wkT = np.ascontiguousarray(w_qkv[H : 2 * H, :].T).reshape(KC, 128, H)
    # U[(2h+a), ci] = sum_d w_out[a, h*D+d] * Wv[h*D+d, ci]
    wo = w_out.reshape(A, NH, D)
    wv = w_qkv[2 * H : 3 * H, :].reshape(NH, D, H)
    U = np.einsum("ahd,hdc->hac", wo, wv, optimize=True).reshape(2 * NH, H)
    uT = np.ascontiguousarray(U.T).reshape(KC, 128, 2 * NH)

    in_maps = []
    for c in range(NCORES):
        rows = slice(c * R, (c + 1) * R)
        hidT = np.ascontiguousarray(hidden_state[rows].T).reshape(KC, 128, R)
        ctx = context_buffer[rows][:, kept, :]  # [R, 7, H]
        ctxT = np.ascontiguousarray(ctx.transpose(1, 2, 0)).reshape(
            W - 1, KC, 128, R
        )
        in_maps.append(
            dict(hidT=hidT, ctxT=ctxT, wqT=wqT, wkT=wkT, uT=uT)
        )
    return in_maps


def kernel(hidden_state, context_buffer, w_qkv, w_out, b_out, context_ptr):
    from concourse.bass_utils import run_bass_kernel_spmd

    nc = _build_nc()
    in_maps = _prep_inputs(
        hidden_state, context_buffer, w_qkv, w_out, b_out, context_ptr
    )
    res = run_bass_kernel_spmd(nc, in_maps, core_ids=list(range(NCORES)))
    out = np.concatenate([r["qout"] for r in res.results], axis=0)
    return (out + np.asarray(b_out, dtype=np.float32)[None, :]).astype(np.float32)

